# revision 34
# baseline (speedup 1.0000x reference)
"""EnsemblePooling (segment mean/max/attention pooling) on 8 Trainium2 cores.

Contract: kernel(**inputs) takes the FULL inputs (x [N,256] f32,
batch [N] i64 sorted, att_w [256,1] f32, att_b [1] f32) and returns the
FULL output [1024, 768] f32 = concat([mean_pool, max_pool, att_pool], -1).

Strategy (all hardcoded, self-contained):
  - core c owns segments [128c, 128(c+1)); nodes sharded by segment;
    every segment's node run is padded to a multiple of 128 so each
    128-node tile belongs to exactly ONE segment (pure-data SPMD).
  - x ships bf16 node-major [128, NT, 256]; loaded in 8-tile supertiles.
  - per tile: PE transposes the two hidden chunks into PSUM; the
    attention scores come from two N=1 matmuls against the evacuated
    transpose (ACT and GPSIMD split the evacuation); DVE folds the
    transposed tile once from PSUM and then tensor-reduces to per-tile
    max columns.  One shared matmul per tile (lhsT = [ones | sigmoid]
    columns) produces sum and attention colsums into per-tile PSUM rows;
    a single one-hot routing matmul pair per 32-tile group accumulates
    them into per-segment rows.
  - epilogue: masked max tournament over the per-tile max columns, then
    one-hot extraction matmuls back to [seg, hidden] layout.
"""

import numpy as np

P = 128
H = 256
G = 1024
CORES = 8
SEGS_PER_CORE = G // CORES  # 128
PAD_X = 0.0  # pads add 0 to colsums; max sees 0, safe (segment max > 0 w.h.p.)
NEG_BIG = -3.0e38  # bf16-representable mask for the max tournament
S_TILES = 8  # node-tiles per DMA super-tile / transpose subgroup
K_TILES = 32  # tiles per colsum K-group (4 subgroups)

_compiled_cache = {}


def _bf16(arr):
    import ml_dtypes

    return np.asarray(arr).astype(ml_dtypes.bfloat16)


def _build_program(NT, NT_real, ks):
    import concourse.bacc as bacc
    import concourse.tile as tile
    from concourse import mybir

    f32 = mybir.dt.float32
    bf16 = mybir.dt.bfloat16
    NG2 = NT // K_TILES
    KC2 = (2 * NT + P - 1) // P  # 128-col chunks over (tile, chunk) max cols
    NC2pad = KC2 * P

    nc = bacc.Bacc("TRN2", target_bir_lowering=False, debug=False)

    x_d = nc.declare_dram_parameter("x", [P, NT, H], bf16, isOutput=False)
    ohs_d = nc.declare_dram_parameter("ohs", [2 * K_TILES, NG2, P], bf16, isOutput=False)
    oha_d = nc.declare_dram_parameter("oha", [2 * K_TILES, NG2, P], bf16, isOutput=False)
    wcol_d = nc.declare_dram_parameter("wcol", [P, 2], bf16, isOutput=False)
    bcol_d = nc.declare_dram_parameter("bcol", [P, 1], f32, isOutput=False)
    ohm0_d = nc.declare_dram_parameter("ohm0", [P, KC2, P], bf16, isOutput=False)
    ohm1_d = nc.declare_dram_parameter("ohm1", [P, KC2, P], bf16, isOutput=False)
    bias_d = {
        k: nc.declare_dram_parameter(f"bias{k}", [P, 2 * NT], bf16, isOutput=False)
        for k in ks
    }
    invcnt_d = nc.declare_dram_parameter("invcnt", [P, 1], f32, isOutput=False)
    out_d = nc.declare_dram_parameter("out", [P, 3 * H], f32, isOutput=True)

    stride = 2 * K_TILES + 2
    with (
        tile.TileContext(nc) as tc,
        tc.tile_pool(name="const", bufs=1) as cpool,
        tc.tile_pool(name="xp", bufs=2) as xpool,
        tc.tile_pool(name="work", bufs=2) as wpool,
        tc.tile_pool(name="xtp", bufs=3) as xtpool,
        tc.tile_pool(name="acc", bufs=1, space="PSUM") as apool,
        tc.tile_pool(name="pst", bufs=2, space="PSUM") as tpool,
        tc.tile_pool(name="csp", bufs=1, space="PSUM") as cspool,
    ):
        # persistent constants; ident + wcol first (they gate the first
        # transposes/scores), bulkier aux tables after
        ident_i = cpool.tile([P, P], mybir.dt.int32)
        nc.gpsimd.iota(ident_i[:], pattern=[[1, P]], base=0, channel_multiplier=-1)
        ident = cpool.tile([P, P], bf16)
        nc.vector.tensor_scalar(
            out=ident[:], in0=ident_i[:], scalar1=0.0, scalar2=None,
            op0=mybir.AluOpType.is_equal,
        )
        # first group's x loads jump the HWDGE queue ahead of the aux tables
        xs0 = []
        for sub in range(S_TILES // 2):
            xsuper = xpool.tile(
                [P, S_TILES, H], bf16, tag=f"xs{sub}", name=f"xs{sub}"
            )
            if sub == 0:
                nc.sync.dma_start(out=xsuper[:, 0:2, :], in_=x_d[:, 0:2, :])
                nc.sync.dma_start(
                    out=xsuper[:, 2:S_TILES, :], in_=x_d[:, 2:S_TILES, :]
                )
            else:
                nc.sync.dma_start(
                    out=xsuper[:],
                    in_=x_d[:, sub * S_TILES : (sub + 1) * S_TILES, :],
                )
            xs0.append(xsuper)
        wcol = cpool.tile([P, 2], bf16)
        nc.sync.dma_start(out=wcol[:], in_=wcol_d[:])
        bcol = cpool.tile([P, 1], f32)
        nc.sync.dma_start(out=bcol[:], in_=bcol_d[:])
        sel = cpool.tile([P, 2, K_TILES * 2 * K_TILES], bf16)
        nc.vector.memset(sel[:], 0.0)
        for b_ in range(2):
            nc.vector.memset(
                sel[:, b_, 0 : 2 * K_TILES * K_TILES : stride], 1.0
            )
        ohs = cpool.tile([2 * K_TILES, NG2, P], bf16)
        nc.sync.dma_start(out=ohs[:], in_=ohs_d[:])
        oha = cpool.tile([2 * K_TILES, NG2, P], bf16)
        nc.sync.dma_start(out=oha[:], in_=oha_d[:])

        # per-tile max columns: col 2t+c = (tile t, hidden chunk c)
        maxc = cpool.tile([P, NC2pad], bf16)
        nc.vector.memset(maxc[:], NEG_BIG)

        psum_sum = apool.tile([P, H], f32)
        psum_att = apool.tile([P, H], f32)

        def emit_colsums(pg2, _unused, pcs_ps, sub, xsuper, nt, last):
            """Emit the colsum matmuls of subgroup `sub` of group pg2."""
            pb = pg2 % 2
            for s in range(nt):
                j = sub * S_TILES + s
                nc.tensor.matmul(
                    pcs_ps[:],
                    lhsT=sel[:, pb, j * 2 * K_TILES : (j + 1) * 2 * K_TILES],
                    rhs=xsuper[:, s, :],
                    start=(j == 0),
                    stop=(last and s == nt - 1),
                )

        def emit_routing(pg2, pcs_ps):
            cs_sb = wpool.tile([2 * K_TILES, H], bf16, tag="cs_sb")
            nc.scalar.copy(cs_sb[:], pcs_ps[:])
            nc.tensor.matmul(
                psum_sum[:], lhsT=ohs[:, pg2, :], rhs=cs_sb[:],
                start=(pg2 == 0), stop=(pg2 == NG2 - 1),
            )
            nc.tensor.matmul(
                psum_att[:], lhsT=oha[:, pg2, :], rhs=cs_sb[:],
                start=(pg2 == 0), stop=(pg2 == NG2 - 1),
            )

        def emit_scores(pg2, sps, sub, xte8, nt=S_TILES):
            for s in range(nt):
                j = sub * S_TILES + s
                for c in range(2):
                    nc.tensor.matmul(
                        sps[:, j : j + 1],
                        lhsT=xte8[:, s, c, :],
                        rhs=wcol[:, c : c + 1],
                        start=(c == 0),
                        stop=(c == 1),
                    )
            # sigmoid for just this subgroup's sigma slots (stride over
            # the flattened sel blocks)
            o0 = 1 + sub * S_TILES * stride
            nc.scalar.activation(
                sel[:, pg2 % 2, o0 : min(o0 + nt * stride, 2 * K_TILES * K_TILES) : stride],
                sps[:, sub * S_TILES : sub * S_TILES + nt],
                mybir.ActivationFunctionType.Sigmoid,
                bias=bcol[:, 0:1],
                scale=1.0,
            )

        # flat software pipeline over 8-tile slots: transposes/evac/fold
        # at slot i, scores+sigmoid for slot i-1, colsums for slot i-2
        NSLOT = (NT_real + S_TILES - 1) // S_TILES  # slots with real tiles
        SPG = K_TILES // S_TILES  # slots per colsum group (4)

        def slot_tiles(i):
            return min(S_TILES, NT_real - i * S_TILES)
        pend_scores = []
        bias_sb = {}
        epi = {}
        xsup = {}
        cs_by_group = {}
        score_by_group = {}
        l1g = None

        def slot_work(i):
            """transposes, evacuation, l1 fold for slot i; tree at group end."""
            nonlocal l1g
            g2, sub = i // SPG, i % SPG
            if g2 == NG2 - 1 and sub == 0:
                # epilogue tables load during the last group's compute
                for k in ks:
                    bias_sb[k] = cpool.tile(
                        [P, 2 * NT], bf16, name=f"bias{k}", tag=f"bias{k}"
                    )
                    nc.sync.dma_start(out=bias_sb[k][:], in_=bias_d[k][:])
                epi["ohm0"] = cpool.tile([P, KC2, P], bf16, name="ohm0")
                nc.sync.dma_start(out=epi["ohm0"][:], in_=ohm0_d[:])
                epi["ohm1"] = cpool.tile([P, KC2, P], bf16, name="ohm1")
                nc.sync.dma_start(out=epi["ohm1"][:], in_=ohm1_d[:])
                epi["invcnt"] = cpool.tile([P, 1], f32, name="invcnt")
                nc.sync.dma_start(out=epi["invcnt"][:], in_=invcnt_d[:])
            ts = i * S_TILES
            nt = slot_tiles(i)
            if i < SPG:
                xsuper = xs0[i]
            else:
                xsuper = xpool.tile(
                    [P, S_TILES, H], bf16, tag=f"xs{i % 4}", name="xsuper"
                )
                nc.sync.dma_start(
                    out=xsuper[:, 0:nt, :], in_=x_d[:, ts : ts + nt, :]
                )
            xsup[i] = xsuper

            psumT = tpool.tile([P, S_TILES, 2, P], bf16, tag="ptg")
            for s in range(nt):
                for c in range(2):
                    nc.tensor.transpose(
                        psumT[:, s, c, :],
                        xsuper[:, s, c * P : (c + 1) * P],
                        ident[:],
                    )

            # evacuate x^T for the score matmuls: ACT 5 tiles, DVE 3
            # (GPSIMD cannot read PSUM)
            na, nd = min(5, nt), nt
            xte8 = xtpool.tile([P, S_TILES, 2, P], bf16, tag="xte")
            nc.scalar.copy(xte8[:, 0:na, :, :], psumT[:, 0:na, :, :])
            if nd > na:
                nc.vector.tensor_copy(
                    xte8[:, na:nd, :, :], psumT[:, na:nd, :, :]
                )

            # max: DVE folds the evacuated transpose into the group
            # buffer; the rest of the fold tree runs once per group
            if sub == 0:
                l1g = wpool.tile(
                    [P, SPG, S_TILES, 2, P // 2], bf16, tag="l1g", name="l1g"
                )
            nc.vector.tensor_tensor(
                out=l1g[:, sub, 0:nt, :, :],
                in0=xte8[:, 0:nt, :, 0 : P // 2],
                in1=xte8[:, 0:nt, :, P // 2 : P],
                op=mybir.AluOpType.max,
            )
            pend_scores.append((g2, sub, xte8, nt))
            if sub == SPG - 1 or i == NSLOT - 1:
                # batched max fold tree over the whole group
                lev = l1g
                w = P // 2
                while w > 2:
                    nxt = wpool.tile(
                        [P, SPG, S_TILES, 2, w // 2], bf16,
                        tag=f"lg{w}", name=f"lg{w}",
                    )
                    nc.vector.tensor_tensor(
                        out=nxt[:],
                        in0=lev[:, :, :, :, 0 : w // 2],
                        in1=lev[:, :, :, :, w // 2 : w],
                        op=mybir.AluOpType.max,
                    )
                    lev = nxt
                    w //= 2
                nc.vector.tensor_tensor(
                    out=maxc[:, 2 * g2 * K_TILES : 2 * (g2 + 1) * K_TILES],
                    in0=lev[:, :, :, :, 0:1],
                    in1=lev[:, :, :, :, 1:2],
                    op=mybir.AluOpType.max,
                )

        def scores_work():
            pg, psub, pxte8, pnt = pend_scores.pop(0)
            if pg not in score_by_group:
                score_by_group[pg] = cspool.tile(
                    [P, K_TILES], f32, tag="score", name="score_ps"
                )
            emit_scores(pg, score_by_group[pg], psub, pxte8, pnt)

        def colsum_work(j):
            """colsums for slot j; routing when j closes its group."""
            pg, psub = j // SPG, j % SPG
            if psub == 0:
                cs_by_group[pg] = cspool.tile(
                    [2 * K_TILES, H], f32, tag="cs", name="cs_ps"
                )
            last = psub == SPG - 1 or j == NSLOT - 1
            emit_colsums(
                pg, None, cs_by_group[pg], psub, xsup.pop(j), slot_tiles(j), last
            )
            if last:
                emit_routing(pg, cs_by_group.pop(pg))
                score_by_group.pop(pg, None)

        import os
        SCORE_LAG = int(os.environ.get("SCORE_LAG", "1"))
        COL_LAG = int(os.environ.get("COL_LAG", "4"))
        for i in range(NSLOT):
            slot_work(i)
            if i >= SCORE_LAG:
                scores_work()
            if i >= COL_LAG:
                colsum_work(i - COL_LAG)

        # ---- epilogue ----
        ohm0 = epi["ohm0"]
        ohm1 = epi["ohm1"]
        invcnt = epi["invcnt"]
        NT_main = NT - K_TILES
        C0 = 2 * NT_main
        KC_pre = C0 // P

        def tour_round(k, lo, hi):
            # masked max with columns shifted by 2k, output range [lo, hi)
            w = hi - 2 * k - lo
            if w <= 0:
                return
            tmp = wpool.tile([P, NC2pad], bf16, tag="tmp_tourn", name="tmp")
            nc.vector.tensor_tensor(
                out=tmp[:, lo : lo + w],
                in0=maxc[:, lo + 2 * k : lo + 2 * k + w],
                in1=bias_sb[k][:, lo : lo + w],
                op=mybir.AluOpType.add,
            )
            nc.vector.tensor_tensor(
                out=maxc[:, lo : lo + w],
                in0=maxc[:, lo : lo + w],
                in1=tmp[:, lo : lo + w],
                op=mybir.AluOpType.max,
            )

        # prefix tournament: the boundary pads guarantee no segment run
        # crosses NT_main, so everything left of C0 is final before the
        # last group's fold tree lands (DVE work, overlaps PE below)
        for k in ks:
            tour_round(k, 0, C0)

        # flush delayed scores and the remaining colsum slots
        while pend_scores:
            scores_work()
        for j in range(NSLOT - COL_LAG, NSLOT):
            colsum_work(j)

        # tail tournament over the last group's columns
        for k in ks:
            tour_round(k, C0, 2 * NT)

        # extraction: transpose max columns chunkwise, one-hot matmuls
        # route them back to [segment, hidden] layout
        psum_max0 = cspool.tile([P, P], f32, tag="score")
        psum_max1 = cspool.tile([P, P], f32, tag="cs")
        for kc in range(KC2):
            ptm = tpool.tile([P, P], bf16, tag="ptg", name="ptm")
            nc.tensor.transpose(
                ptm[:], maxc[:, kc * P : (kc + 1) * P], ident[:]
            )
            tmt = wpool.tile([P, P], bf16, tag="tmt", name="tmt")
            nc.scalar.copy(tmt[:], ptm[:])
            nc.tensor.matmul(
                psum_max0[:], lhsT=ohm0[:, kc, :], rhs=tmt[:],
                start=(kc == 0), stop=(kc == KC2 - 1),
            )
            nc.tensor.matmul(
                psum_max1[:], lhsT=ohm1[:, kc, :], rhs=tmt[:],
                start=(kc == 0), stop=(kc == KC2 - 1),
            )

        out_sb = cpool.tile([P, 3 * H], f32)
        nc.scalar.mul(out_sb[:, 0:H], psum_sum[:], invcnt[:, 0:1])
        nc.vector.tensor_copy(out_sb[:, H : H + P], psum_max0[:])
        nc.vector.tensor_copy(out_sb[:, H + P : 2 * H], psum_max1[:])
        nc.scalar.copy(out_sb[:, 2 * H : 3 * H], psum_att[:])
        nc.sync.dma_start(out=out_d[:], in_=out_sb[:])

    nc.finalize()
    return nc


def _prepare_inputs(x, batch, att_w, att_b):
    """Host-side sharding/index preprocessing. Returns (in_maps, NT, ks)."""
    N = x.shape[0]
    assert x.shape == (N, H) and batch.shape == (N,)

    counts = np.bincount(batch, minlength=G).astype(np.int64)
    starts = np.concatenate([[0], np.cumsum(counts)])
    tiles_per_seg = (counts + P - 1) // P  # 0 for empty segments

    core_nt = [
        int(tiles_per_seg[c * SEGS_PER_CORE : (c + 1) * SEGS_PER_CORE].sum())
        for c in range(CORES)
    ]
    # Iterate NT so that each core can insert pad tiles to keep any
    # segment run from crossing the last-group boundary (NT - K_TILES);
    # the epilogue prefix work relies on that alignment.
    NT = ((max(max(core_nt), 2) + K_TILES - 1) // K_TILES) * K_TILES
    for _ in range(4):
        NT_main = NT - K_TILES
        worst = 0
        for c in range(CORES):
            runs = tiles_per_seg[c * SEGS_PER_CORE : (c + 1) * SEGS_PER_CORE]
            t = 0
            pads = 0
            for r in runs:
                r = int(r)
                if t < NT_main and t + r > NT_main:
                    pads = NT_main - t
                    t = NT_main
                t += r
            worst = max(worst, t)
        newNT = ((max(worst, 2) + K_TILES - 1) // K_TILES) * K_TILES
        if newNT == NT:
            break
        NT = newNT
    NG2 = NT // K_TILES
    KC2 = (2 * NT + P - 1) // P
    NC2pad = KC2 * P

    # real per-core tile counts including boundary pads
    NT_main = NT - K_TILES
    real_counts = []
    for c in range(CORES):
        runs = tiles_per_seg[c * SEGS_PER_CORE : (c + 1) * SEGS_PER_CORE]
        t = 0
        for r in runs:
            r = int(r)
            if r == 0:
                continue
            if t < NT_main and t + r > NT_main:
                t = NT_main
            t += r
        real_counts.append(t)
    NT_real = max(max(real_counts), 2)

    max_run = int(tiles_per_seg.max())
    ks = []
    k = 1
    while k < max(max_run, 1):
        ks.append(k)
        k *= 2
    if not ks:
        ks = [1]

    wcol = _bf16(att_w.reshape(2, P).T)
    bcol = np.full((P, 1), att_b[0], dtype=np.float32)


    in_maps = []
    for c in range(CORES):
        g0 = c * SEGS_PER_CORE
        flat_x = np.full((NT * P, H), PAD_X, dtype=np.float32)
        seg_of_tile = np.full((NT,), -1, dtype=np.int64)
        ohm0 = np.zeros((NC2pad, P), dtype=np.float32)
        ohm1 = np.zeros((NC2pad, P), dtype=np.float32)

        t = 0
        NT_main = NT - K_TILES
        for gl in range(SEGS_PER_CORE):
            g = g0 + gl
            cnt = int(counts[g])
            if cnt == 0:
                continue
            ntg = int(tiles_per_seg[g])
            if t < NT_main and t + ntg > NT_main:
                t = NT_main  # pad tiles keep the run inside the last group
            n0 = int(starts[g])
            flat_x[t * P : t * P + cnt] = x[n0 : n0 + cnt]
            seg_of_tile[t : t + ntg] = gl
            ohm0[2 * t, gl] = 1.0
            ohm1[2 * t + 1, gl] = 1.0
            t += ntg

        x_dev = _bf16(flat_x.reshape(NT, P, H).transpose(1, 0, 2))

        # routing one-hots: row 2j -> segment of tile (sum), 2j+1 (att)
        ohs = np.zeros((2 * K_TILES, NG2, P), np.float32)
        oha = np.zeros((2 * K_TILES, NG2, P), np.float32)
        for tt in range(NT):
            gl = seg_of_tile[tt]
            if gl < 0:
                continue
            g2, j = tt // K_TILES, tt % K_TILES
            ohs[2 * j, g2, gl] = 1.0
            oha[2 * j + 1, g2, gl] = 1.0

        m = {
            "x": np.ascontiguousarray(x_dev),
            "ohs": _bf16(ohs),
            "oha": _bf16(oha),
            "wcol": wcol,
            "bcol": bcol,
            "ohm0": _bf16(
                np.ascontiguousarray(ohm0.reshape(KC2, P, P).transpose(1, 0, 2))
            ),
            "ohm1": _bf16(
                np.ascontiguousarray(ohm1.reshape(KC2, P, P).transpose(1, 0, 2))
            ),
            "invcnt": (
                1.0
                / np.maximum(counts[g0 : g0 + SEGS_PER_CORE], 1).astype(np.float32)
            ).reshape(P, 1),
        }
        for k in ks:
            bias = np.full((P, 2 * NT), NEG_BIG, dtype=np.float32)
            same = (seg_of_tile[k:] == seg_of_tile[:-k]) & (seg_of_tile[:-k] >= 0)
            same2 = np.repeat(same, 2)
            bias[:, : 2 * (NT - k)][:, same2] = 0.0
            m[f"bias{k}"] = _bf16(bias)
        in_maps.append(m)

    return in_maps, NT, NT_real, ks


def kernel(x, batch, att_w, att_b):
    x = np.ascontiguousarray(np.asarray(x, dtype=np.float32))
    batch = np.asarray(batch).astype(np.int64)
    att_w = np.asarray(att_w, dtype=np.float32).reshape(H, 1)
    att_b = np.asarray(att_b, dtype=np.float32).reshape(1)

    in_maps, NT, NT_real, ks = _prepare_inputs(x, batch, att_w, att_b)

    key = (NT, NT_real, tuple(ks))
    if key not in _compiled_cache:
        _compiled_cache[key] = _build_program(NT, NT_real, ks)
    nc = _compiled_cache[key]

    from concourse.bass_utils import run_bass_kernel_spmd

    res = run_bass_kernel_spmd(nc, in_maps, list(range(CORES)))
    global _last_result
    _last_result = res
    out = np.concatenate(
        [np.asarray(res.results[c]["out"]) for c in range(CORES)], axis=0
    )
    return out.astype(np.float32)


# revision 41
# speedup vs baseline: 1.0050x; 1.0050x over previous
"""EnsemblePooling (segment mean/max/attention pooling) on 8 Trainium2 cores.

Contract: kernel(**inputs) takes the FULL inputs (x [N,256] f32,
batch [N] i64 sorted, att_w [256,1] f32, att_b [1] f32) and returns the
FULL output [1024, 768] f32 = concat([mean_pool, max_pool, att_pool], -1).

Strategy (all hardcoded, self-contained):
  - core c owns segments [128c, 128(c+1)); nodes sharded by segment;
    every segment's node run is padded to a multiple of 128 so each
    128-node tile belongs to exactly ONE segment (pure-data SPMD).
  - x ships bf16 node-major [128, NT, 256]; loaded in 8-tile supertiles.
  - per tile: PE transposes the two hidden chunks into PSUM; the
    attention scores come from two N=1 matmuls against the evacuated
    transpose (ACT and GPSIMD split the evacuation); DVE folds the
    transposed tile once from PSUM and then tensor-reduces to per-tile
    max columns.  One shared matmul per tile (lhsT = [ones | sigmoid]
    columns) produces sum and attention colsums into per-tile PSUM rows;
    a single one-hot routing matmul pair per 32-tile group accumulates
    them into per-segment rows.
  - epilogue: masked max tournament over the per-tile max columns, then
    one-hot extraction matmuls back to [seg, hidden] layout.
"""

import numpy as np

P = 128
H = 256
G = 1024
CORES = 8
SEGS_PER_CORE = G // CORES  # 128
PAD_X = 0.0  # pads add 0 to colsums; max sees 0, safe (segment max > 0 w.h.p.)
NEG_BIG = -3.0e38  # bf16-representable mask for the max tournament
S_TILES = 8  # node-tiles per DMA super-tile / transpose subgroup
K_TILES = 32  # tiles per colsum K-group (4 subgroups)

_compiled_cache = {}


def _bf16(arr):
    import ml_dtypes

    return np.asarray(arr).astype(ml_dtypes.bfloat16)


def _build_program(NT, NT_real, ks):
    import concourse.bacc as bacc
    import concourse.tile as tile
    from concourse import mybir

    f32 = mybir.dt.float32
    bf16 = mybir.dt.bfloat16
    NG2 = NT // K_TILES
    KC2 = (2 * NT + P - 1) // P  # 128-col chunks over (tile, chunk) max cols
    NC2pad = KC2 * P

    nc = bacc.Bacc("TRN2", target_bir_lowering=False, debug=False)

    x_d = nc.declare_dram_parameter("x", [P, NT, H], bf16, isOutput=False)
    ohs_d = nc.declare_dram_parameter("ohs", [2 * K_TILES, NG2, P], bf16, isOutput=False)
    oha_d = nc.declare_dram_parameter("oha", [2 * K_TILES, NG2, P], bf16, isOutput=False)
    wcol_d = nc.declare_dram_parameter("wcol", [P, 2], bf16, isOutput=False)
    bcol_d = nc.declare_dram_parameter("bcol", [P, 1], f32, isOutput=False)
    ohm0_d = nc.declare_dram_parameter("ohm0", [P, KC2, P], bf16, isOutput=False)
    ohm1_d = nc.declare_dram_parameter("ohm1", [P, KC2, P], bf16, isOutput=False)
    bias_d = {
        k: nc.declare_dram_parameter(f"bias{k}", [P, 2 * NT], bf16, isOutput=False)
        for k in ks
    }
    invcnt_d = nc.declare_dram_parameter("invcnt", [P, 1], f32, isOutput=False)
    out_d = nc.declare_dram_parameter("out", [P, 3 * H], f32, isOutput=True)

    stride = 2 * K_TILES + 2
    with (
        tile.TileContext(nc) as tc,
        tc.tile_pool(name="const", bufs=1) as cpool,
        tc.tile_pool(name="xp", bufs=3) as xpool,
        tc.tile_pool(name="work", bufs=3) as wpool,
        tc.tile_pool(name="xtp", bufs=3) as xtpool,
        tc.tile_pool(name="acc", bufs=1, space="PSUM") as apool,
        tc.tile_pool(name="pst", bufs=2, space="PSUM") as tpool,
        tc.tile_pool(name="csp", bufs=1, space="PSUM") as cspool,
    ):
        # persistent constants; ident + wcol first (they gate the first
        # transposes/scores), bulkier aux tables after
        ident_i = cpool.tile([P, P], mybir.dt.int32)
        nc.gpsimd.iota(ident_i[:], pattern=[[1, P]], base=0, channel_multiplier=-1)
        ident = cpool.tile([P, P], bf16)
        nc.vector.tensor_scalar(
            out=ident[:], in0=ident_i[:], scalar1=0.0, scalar2=None,
            op0=mybir.AluOpType.is_equal,
        )
        # first group's x loads jump the HWDGE queue ahead of the aux tables
        xs0 = []
        for sub in range(S_TILES // 2):
            xsuper = xpool.tile(
                [P, S_TILES, H], bf16, tag=f"xs{sub}", name=f"xs{sub}"
            )
            ts0 = sub * S_TILES
            if sub < 2:
                nc.sync.dma_start(
                    out=xsuper[:, 0:2, :], in_=x_d[:, ts0 : ts0 + 2, :]
                )
                nc.sync.dma_start(
                    out=xsuper[:, 2:S_TILES, :],
                    in_=x_d[:, ts0 + 2 : ts0 + S_TILES, :],
                )
            else:
                nc.sync.dma_start(
                    out=xsuper[:], in_=x_d[:, ts0 : ts0 + S_TILES, :]
                )
            xs0.append(xsuper)
        wcol = cpool.tile([P, 2], bf16)
        nc.sync.dma_start(out=wcol[:], in_=wcol_d[:])
        bcol = cpool.tile([P, 1], f32)
        nc.sync.dma_start(out=bcol[:], in_=bcol_d[:])
        sel = cpool.tile([P, 2, K_TILES * 2 * K_TILES], bf16)
        nc.vector.memset(sel[:], 0.0)
        for b_ in range(2):
            nc.vector.memset(
                sel[:, b_, 0 : 2 * K_TILES * K_TILES : stride], 1.0
            )
        ohs = cpool.tile([2 * K_TILES, NG2, P], bf16)
        nc.sync.dma_start(out=ohs[:], in_=ohs_d[:])
        oha = cpool.tile([2 * K_TILES, NG2, P], bf16)
        nc.sync.dma_start(out=oha[:], in_=oha_d[:])

        # per-tile max columns: col 2t+c = (tile t, hidden chunk c)
        maxc = cpool.tile([P, NC2pad], bf16)
        nc.vector.memset(maxc[:], NEG_BIG)

        psum_sum = apool.tile([P, H], f32)
        psum_att = apool.tile([P, H], f32)

        def emit_colsums(pg2, _unused, pcs_ps, sub, xsuper, nt, last):
            """Emit the colsum matmuls of subgroup `sub` of group pg2."""
            pb = pg2 % 2
            for s in range(nt):
                j = sub * S_TILES + s
                nc.tensor.matmul(
                    pcs_ps[:],
                    lhsT=sel[:, pb, j * 2 * K_TILES : (j + 1) * 2 * K_TILES],
                    rhs=xsuper[:, s, :],
                    start=(j == 0),
                    stop=(last and s == nt - 1),
                )

        def emit_routing(pg2, pcs_ps):
            cs_sb = wpool.tile([2 * K_TILES, H], bf16, tag="cs_sb")
            nc.scalar.copy(cs_sb[:], pcs_ps[:])
            nc.tensor.matmul(
                psum_sum[:], lhsT=ohs[:, pg2, :], rhs=cs_sb[:],
                start=(pg2 == 0), stop=(pg2 == NG2 - 1),
            )
            nc.tensor.matmul(
                psum_att[:], lhsT=oha[:, pg2, :], rhs=cs_sb[:],
                start=(pg2 == 0), stop=(pg2 == NG2 - 1),
            )

        def emit_scores(pg2, sps, sub, xte8, nt=S_TILES):
            for s in range(nt):
                j = sub * S_TILES + s
                for c in range(2):
                    nc.tensor.matmul(
                        sps[:, j : j + 1],
                        lhsT=xte8[:, s, c, :],
                        rhs=wcol[:, c : c + 1],
                        start=(c == 0),
                        stop=(c == 1),
                    )
            # sigmoid for just this subgroup's sigma slots (stride over
            # the flattened sel blocks)
            o0 = 1 + sub * S_TILES * stride
            nc.scalar.activation(
                sel[:, pg2 % 2, o0 : min(o0 + nt * stride, 2 * K_TILES * K_TILES) : stride],
                sps[:, sub * S_TILES : sub * S_TILES + nt],
                mybir.ActivationFunctionType.Sigmoid,
                bias=bcol[:, 0:1],
                scale=1.0,
            )

        # flat software pipeline over 8-tile slots: transposes/evac/fold
        # at slot i, scores+sigmoid for slot i-1, colsums for slot i-2
        NSLOT = (NT_real + S_TILES - 1) // S_TILES  # slots with real tiles
        SPG = K_TILES // S_TILES  # slots per colsum group (4)

        def slot_tiles(i):
            return min(S_TILES, NT_real - i * S_TILES)
        pend_scores = []
        bias_sb = {}
        epi = {}
        xsup = {}
        cs_by_group = {}
        score_by_group = {}
        l1g = None

        def slot_work(i):
            """transposes, evacuation, l1 fold for slot i; tree at group end."""
            nonlocal l1g
            g2, sub = i // SPG, i % SPG
            if g2 == NG2 - 1 and sub == 0:
                # epilogue tables load during the last group's compute
                for k in ks:
                    bias_sb[k] = cpool.tile(
                        [P, 2 * NT], bf16, name=f"bias{k}", tag=f"bias{k}"
                    )
                    nc.sync.dma_start(out=bias_sb[k][:], in_=bias_d[k][:])
                epi["ohm0"] = cpool.tile([P, KC2, P], bf16, name="ohm0")
                nc.sync.dma_start(out=epi["ohm0"][:], in_=ohm0_d[:])
                epi["ohm1"] = cpool.tile([P, KC2, P], bf16, name="ohm1")
                nc.sync.dma_start(out=epi["ohm1"][:], in_=ohm1_d[:])
                epi["invcnt"] = cpool.tile([P, 1], f32, name="invcnt")
                nc.sync.dma_start(out=epi["invcnt"][:], in_=invcnt_d[:])
            ts = i * S_TILES
            nt = slot_tiles(i)
            if i < SPG:
                xsuper = xs0[i]
            else:
                xsuper = xpool.tile(
                    [P, S_TILES, H], bf16, tag=f"xs{i % 4}", name="xsuper"
                )
                nc.sync.dma_start(
                    out=xsuper[:, 0:nt, :], in_=x_d[:, ts : ts + nt, :]
                )
            xsup[i] = xsuper

            psumT = tpool.tile([P, S_TILES, 2, P], bf16, tag="ptg")
            na, nd = min(5, nt), nt
            for s in range(nt):
                for c in range(2):
                    nc.tensor.transpose(
                        psumT[:, s, c, :],
                        xsuper[:, s, c * P : (c + 1) * P],
                        ident[:],
                    )

            # evacuate x^T for the score matmuls: ACT 5 tiles, DVE 3
            # (GPSIMD cannot read PSUM)
            xte8 = xtpool.tile([P, S_TILES, 2, P], bf16, tag="xte")
            nc.scalar.copy(xte8[:, 0:na, :, :], psumT[:, 0:na, :, :])
            if nd > na:
                nc.vector.tensor_copy(
                    xte8[:, na:nd, :, :], psumT[:, na:nd, :, :]
                )

            # max: DVE folds the evacuated transpose into the group
            # buffer; the rest of the fold tree runs once per group
            if sub == 0:
                l1g = wpool.tile(
                    [P, SPG, S_TILES, 2, P // 2], bf16, tag="l1g", name="l1g"
                )
            nc.vector.tensor_tensor(
                out=l1g[:, sub, 0:nt, :, :],
                in0=xte8[:, 0:nt, :, 0 : P // 2],
                in1=xte8[:, 0:nt, :, P // 2 : P],
                op=mybir.AluOpType.max,
            )
            pend_scores.append((g2, sub, xte8, nt))
            if sub == SPG - 1 or i == NSLOT - 1:
                # batched max fold tree over the whole group
                lev = l1g
                w = P // 2
                while w > 2:
                    nxt = wpool.tile(
                        [P, SPG, S_TILES, 2, w // 2], bf16,
                        tag=f"lg{w}", name=f"lg{w}",
                    )
                    nc.vector.tensor_tensor(
                        out=nxt[:],
                        in0=lev[:, :, :, :, 0 : w // 2],
                        in1=lev[:, :, :, :, w // 2 : w],
                        op=mybir.AluOpType.max,
                    )
                    lev = nxt
                    w //= 2
                nc.vector.tensor_tensor(
                    out=maxc[:, 2 * g2 * K_TILES : 2 * (g2 + 1) * K_TILES],
                    in0=lev[:, :, :, :, 0:1],
                    in1=lev[:, :, :, :, 1:2],
                    op=mybir.AluOpType.max,
                )

        def scores_work():
            pg, psub, pxte8, pnt = pend_scores.pop(0)
            if pg not in score_by_group:
                score_by_group[pg] = cspool.tile(
                    [P, K_TILES], f32, tag="score", name="score_ps"
                )
            emit_scores(pg, score_by_group[pg], psub, pxte8, pnt)

        def colsum_work(j):
            """colsums for slot j; routing when j closes its group."""
            pg, psub = j // SPG, j % SPG
            if psub == 0:
                cs_by_group[pg] = cspool.tile(
                    [2 * K_TILES, H], f32, tag="cs", name="cs_ps"
                )
            last = psub == SPG - 1 or j == NSLOT - 1
            emit_colsums(
                pg, None, cs_by_group[pg], psub, xsup.pop(j), slot_tiles(j), last
            )
            if last:
                emit_routing(pg, cs_by_group.pop(pg))
                score_by_group.pop(pg, None)

        SCORE_LAG = 1  # scores trail the transpose/evac slot by one
        COL_LAG = 4  # colsums trail by four slots (sigma ready, PE fed)
        for i in range(NSLOT):
            slot_work(i)
            if i >= COL_LAG:
                colsum_work(i - COL_LAG)
            if i >= SCORE_LAG:
                scores_work()

        # ---- epilogue ----
        ohm0 = epi["ohm0"]
        ohm1 = epi["ohm1"]
        invcnt = epi["invcnt"]
        NT_main = NT - K_TILES
        C0 = 2 * NT_main
        KC_pre = C0 // P

        def tour_round(k, lo, hi):
            # masked max with columns shifted by 2k, output range [lo, hi)
            w = hi - 2 * k - lo
            if w <= 0:
                return
            tmp = wpool.tile([P, NC2pad], bf16, tag="tmp_tourn", name="tmp")
            nc.vector.tensor_tensor(
                out=tmp[:, lo : lo + w],
                in0=maxc[:, lo + 2 * k : lo + 2 * k + w],
                in1=bias_sb[k][:, lo : lo + w],
                op=mybir.AluOpType.add,
            )
            nc.vector.tensor_tensor(
                out=maxc[:, lo : lo + w],
                in0=maxc[:, lo : lo + w],
                in1=tmp[:, lo : lo + w],
                op=mybir.AluOpType.max,
            )

        # prefix tournament: the boundary pads guarantee no segment run
        # crosses NT_main, so everything left of C0 is final before the
        # last group's fold tree lands (DVE work, overlaps PE below)
        for k in ks:
            tour_round(k, 0, C0)

        # flush delayed scores and the remaining colsum slots
        while pend_scores:
            scores_work()
        for j in range(NSLOT - COL_LAG, NSLOT):
            colsum_work(j)

        # tail tournament over the last group's columns
        for k in ks:
            tour_round(k, C0, 2 * NT)

        # extraction: transpose max columns chunkwise, one-hot matmuls
        # route them back to [segment, hidden] layout
        psum_max0 = cspool.tile([P, P], f32, tag="score")
        psum_max1 = cspool.tile([P, P], f32, tag="cs")
        for kc in range(KC2):
            ptm = tpool.tile([P, P], bf16, tag="ptg", name="ptm")
            nc.tensor.transpose(
                ptm[:], maxc[:, kc * P : (kc + 1) * P], ident[:]
            )
            tmt = wpool.tile([P, P], bf16, tag="tmt", name="tmt")
            nc.scalar.copy(tmt[:], ptm[:])
            nc.tensor.matmul(
                psum_max0[:], lhsT=ohm0[:, kc, :], rhs=tmt[:],
                start=(kc == 0), stop=(kc == KC2 - 1),
            )
            nc.tensor.matmul(
                psum_max1[:], lhsT=ohm1[:, kc, :], rhs=tmt[:],
                start=(kc == 0), stop=(kc == KC2 - 1),
            )

        out_sb = cpool.tile([P, 3 * H], f32)
        nc.scalar.mul(out_sb[:, 0:H], psum_sum[:], invcnt[:, 0:1])
        nc.vector.tensor_copy(out_sb[:, H : H + P], psum_max0[:])
        nc.vector.tensor_copy(out_sb[:, H + P : 2 * H], psum_max1[:])
        nc.sync.dma_start(out=out_d[:, 0 : 2 * H], in_=out_sb[:, 0 : 2 * H])
        nc.scalar.copy(out_sb[:, 2 * H : 3 * H], psum_att[:])
        nc.sync.dma_start(
            out=out_d[:, 2 * H : 3 * H], in_=out_sb[:, 2 * H : 3 * H]
        )

    nc.finalize()
    return nc


def _prepare_inputs(x, batch, att_w, att_b):
    """Host-side sharding/index preprocessing. Returns (in_maps, NT, ks)."""
    N = x.shape[0]
    assert x.shape == (N, H) and batch.shape == (N,)

    counts = np.bincount(batch, minlength=G).astype(np.int64)
    starts = np.concatenate([[0], np.cumsum(counts)])
    tiles_per_seg = (counts + P - 1) // P  # 0 for empty segments

    core_nt = [
        int(tiles_per_seg[c * SEGS_PER_CORE : (c + 1) * SEGS_PER_CORE].sum())
        for c in range(CORES)
    ]
    # Iterate NT so that each core can insert pad tiles to keep any
    # segment run from crossing the last-group boundary (NT - K_TILES);
    # the epilogue prefix work relies on that alignment.
    NT = ((max(max(core_nt), 2) + K_TILES - 1) // K_TILES) * K_TILES
    for _ in range(4):
        NT_main = NT - K_TILES
        worst = 0
        for c in range(CORES):
            runs = tiles_per_seg[c * SEGS_PER_CORE : (c + 1) * SEGS_PER_CORE]
            t = 0
            pads = 0
            for r in runs:
                r = int(r)
                if t < NT_main and t + r > NT_main:
                    pads = NT_main - t
                    t = NT_main
                t += r
            worst = max(worst, t)
        newNT = ((max(worst, 2) + K_TILES - 1) // K_TILES) * K_TILES
        if newNT == NT:
            break
        NT = newNT
    NG2 = NT // K_TILES
    KC2 = (2 * NT + P - 1) // P
    NC2pad = KC2 * P

    # real per-core tile counts including boundary pads
    NT_main = NT - K_TILES
    real_counts = []
    for c in range(CORES):
        runs = tiles_per_seg[c * SEGS_PER_CORE : (c + 1) * SEGS_PER_CORE]
        t = 0
        for r in runs:
            r = int(r)
            if r == 0:
                continue
            if t < NT_main and t + r > NT_main:
                t = NT_main
            t += r
        real_counts.append(t)
    NT_real = max(max(real_counts), 2)

    max_run = int(tiles_per_seg.max())
    ks = []
    k = 1
    while k < max(max_run, 1):
        ks.append(k)
        k *= 2
    if not ks:
        ks = [1]

    wcol = _bf16(att_w.reshape(2, P).T)
    bcol = np.full((P, 1), att_b[0], dtype=np.float32)


    in_maps = []
    for c in range(CORES):
        g0 = c * SEGS_PER_CORE
        flat_x = np.full((NT * P, H), PAD_X, dtype=np.float32)
        seg_of_tile = np.full((NT,), -1, dtype=np.int64)
        ohm0 = np.zeros((NC2pad, P), dtype=np.float32)
        ohm1 = np.zeros((NC2pad, P), dtype=np.float32)

        t = 0
        NT_main = NT - K_TILES
        for gl in range(SEGS_PER_CORE):
            g = g0 + gl
            cnt = int(counts[g])
            if cnt == 0:
                continue
            ntg = int(tiles_per_seg[g])
            if t < NT_main and t + ntg > NT_main:
                t = NT_main  # pad tiles keep the run inside the last group
            n0 = int(starts[g])
            flat_x[t * P : t * P + cnt] = x[n0 : n0 + cnt]
            seg_of_tile[t : t + ntg] = gl
            ohm0[2 * t, gl] = 1.0
            ohm1[2 * t + 1, gl] = 1.0
            t += ntg

        x_dev = _bf16(flat_x.reshape(NT, P, H).transpose(1, 0, 2))

        # routing one-hots: row 2j -> segment of tile (sum), 2j+1 (att)
        ohs = np.zeros((2 * K_TILES, NG2, P), np.float32)
        oha = np.zeros((2 * K_TILES, NG2, P), np.float32)
        for tt in range(NT):
            gl = seg_of_tile[tt]
            if gl < 0:
                continue
            g2, j = tt // K_TILES, tt % K_TILES
            ohs[2 * j, g2, gl] = 1.0
            oha[2 * j + 1, g2, gl] = 1.0

        m = {
            "x": np.ascontiguousarray(x_dev),
            "ohs": _bf16(ohs),
            "oha": _bf16(oha),
            "wcol": wcol,
            "bcol": bcol,
            "ohm0": _bf16(
                np.ascontiguousarray(ohm0.reshape(KC2, P, P).transpose(1, 0, 2))
            ),
            "ohm1": _bf16(
                np.ascontiguousarray(ohm1.reshape(KC2, P, P).transpose(1, 0, 2))
            ),
            "invcnt": (
                1.0
                / np.maximum(counts[g0 : g0 + SEGS_PER_CORE], 1).astype(np.float32)
            ).reshape(P, 1),
        }
        for k in ks:
            bias = np.full((P, 2 * NT), NEG_BIG, dtype=np.float32)
            same = (seg_of_tile[k:] == seg_of_tile[:-k]) & (seg_of_tile[:-k] >= 0)
            same2 = np.repeat(same, 2)
            bias[:, : 2 * (NT - k)][:, same2] = 0.0
            m[f"bias{k}"] = _bf16(bias)
        in_maps.append(m)

    return in_maps, NT, NT_real, ks


def kernel(x, batch, att_w, att_b):
    x = np.ascontiguousarray(np.asarray(x, dtype=np.float32))
    batch = np.asarray(batch).astype(np.int64)
    att_w = np.asarray(att_w, dtype=np.float32).reshape(H, 1)
    att_b = np.asarray(att_b, dtype=np.float32).reshape(1)

    in_maps, NT, NT_real, ks = _prepare_inputs(x, batch, att_w, att_b)

    key = (NT, NT_real, tuple(ks))
    if key not in _compiled_cache:
        _compiled_cache[key] = _build_program(NT, NT_real, ks)
    nc = _compiled_cache[key]

    from concourse.bass_utils import run_bass_kernel_spmd

    res = run_bass_kernel_spmd(nc, in_maps, list(range(CORES)))
    global _last_result
    _last_result = res
    out = np.concatenate(
        [np.asarray(res.results[c]["out"]) for c in range(CORES)], axis=0
    )
    return out.astype(np.float32)
